# revision 14
# baseline (speedup 1.0000x reference)
"""Multi-head attention (B=2, S=2048, D=1024, H=16) on 8 Trainium2 NeuronCores.

Strategy: tensor-parallel over heads — 2 heads per core. Each core:
  - computes Q^T/K^T/V for its 2 heads from the full token stream
    (bf16 matmuls with fp32 PSUM accumulation; bf16 keeps the PE HAM
    activity monitor warm at 2.4 GHz — fp32/f32r matmuls run in PE
    transpose-mode which HAM ignores, throttling the clock to 1.2 GHz),
  - computes transposed attention scores  scoresT[k, q] = (K Q^T)/8 per
    (batch, head), exponentiates on the scalar engine (no max-subtraction:
    scores are O(1) for this distribution, exp cannot overflow),
  - gets softmax denominators for free from a ones-column appended to the
    V stationary during the A@V matmul accumulation,
  - ships the raw bf16 exp tiles in [k, q] layout plus the per-q
    reciprocal row sums; the host transposes and normalizes,
  - computes its partial fc projection out_part = Wfc[:, own_cols] @ attn_out
    (the tensor-parallel all-reduce is done on the host at unshard time).

Host side: shard/pre-transpose/bf16-cast inputs, run SPMD on 8 cores, then
  weights[b, 2c+h] = expT_c[2b+h].T * inv_c[2b+h][:, None]
  out = (sum_c fc_part_c).T + bfc.
"""

import numpy as np
import ml_dtypes

B = 2
S = 2048
D = 1024
NH = 16
DH = 64
T = B * S               # 4096 tokens
N_CORES = 8
HPC = NH // N_CORES     # 2 heads per core
EC = HPC * DH           # 128 embedding cols per core
QHALF = 512             # q positions per inner attention pass
NKT = S // 128          # 16 k-tiles per batch

_NC_CACHE = None


def _build():
    from concourse import bacc, mybir
    import concourse.tile as tile
    from concourse.bass import ts
    from concourse.masks import make_identity

    f32 = mybir.dt.float32
    bf16 = mybir.dt.bfloat16
    EXP = mybir.ActivationFunctionType.Exp

    nc = bacc.Bacc("TRN2", target_bir_lowering=False, debug=False,
                   num_devices=N_CORES)

    xT = nc.dram_tensor("xT", [D, T], bf16, kind="ExternalInput").ap()
    wqT = nc.dram_tensor("wqT", [D, EC], bf16, kind="ExternalInput").ap()
    wkT = nc.dram_tensor("wkT", [D, EC], bf16, kind="ExternalInput").ap()
    wvT = nc.dram_tensor("wvT", [D, EC], bf16, kind="ExternalInput").ap()
    bqv = nc.dram_tensor("bqv", [EC, 3], f32, kind="ExternalInput").ap()
    wfcT = nc.dram_tensor("wfcT", [HPC, DH, D], bf16,
                          kind="ExternalInput").ap()

    # raw exp(scores/8) in transposed [k, q] layout: [b*2+h][k][q]
    wT_out = nc.dram_tensor("wT_out", [B * HPC, S, S], bf16,
                            kind="ExternalOutput").ap()
    # reciprocal softmax denominators per q: [b*2+h][q]
    inv_out = nc.dram_tensor("inv_out", [B * HPC, S], f32,
                             kind="ExternalOutput").ap()
    fc_out = nc.dram_tensor("fc_out", [D, T], f32, kind="ExternalOutput").ap()

    xT_r = xT.rearrange("(a p) t -> p a t", p=128)      # [128, 8, T]
    wqT_r = wqT.rearrange("(a p) e -> p a e", p=128)    # [128, 8, EC]
    wkT_r = wkT.rearrange("(a p) e -> p a e", p=128)
    wvT_r = wvT.rearrange("(a p) e -> p a e", p=128)
    wfcT_r = wfcT.rearrange("h e o -> e h o")           # [64, 2, D]

    with tile.TileContext(nc) as tc:
        with (
            tc.tile_pool(name="const", bufs=1) as constp,
            tc.tile_pool(name="persist", bufs=1) as persist,
        ):
            # ---- constants ----
            wq_sb = constp.tile([128, 8, EC], bf16)
            wk_sb = constp.tile([128, 8, EC], bf16)
            wv_sb = constp.tile([128, 8, EC], bf16)
            nc.sync.dma_start(out=wq_sb, in_=wqT_r)
            nc.sync.dma_start(out=wk_sb, in_=wkT_r)
            nc.sync.dma_start(out=wv_sb, in_=wvT_r)
            bias_sb = constp.tile([EC, 3], f32)
            nc.sync.dma_start(out=bias_sb, in_=bqv)
            wfc_sb = constp.tile([DH, HPC, D], bf16)
            nc.sync.dma_start(out=wfc_sb, in_=wfcT_r)
            ident = constp.tile([128, 128], bf16)
            make_identity(nc, ident[:, :])

            # ---- persistent activations ----
            QT = persist.tile([EC, T], bf16)            # [2h*64, t]
            KT = persist.tile([EC, T], bf16)
            # V per global k-tile and head, with a ones column for the
            # softmax denominators: [k(128), kt(32), h(2), DH+1]
            V_sb = persist.tile([128, 2 * NKT, HPC, DH + 1], bf16)
            # attention output (normalized), per (b, h): rows 0..63
            outT = persist.tile([DH, B * HPC, S], bf16)

            nc.vector.memset(V_sb[:, :, :, DH:DH + 1], 1.0)

            # ============ QKV, then attention with fc interleaved ============
            import contextlib
            with contextlib.ExitStack() as phase1:
                xload = phase1.enter_context(
                    tc.tile_pool(name="xload", bufs=3))
                vt_tmp = phase1.enter_context(
                    tc.tile_pool(name="vt_tmp", bufs=2))
                qkvtr_ps = phase1.enter_context(
                    tc.tile_pool(name="qkvtr_ps", bufs=3, space="PSUM"))
                pools = {}

                def emit_qkv_chunk(tci):
                    xc = xload.tile([128, 8, 512], bf16)
                    nc.sync.dma_start(out=xc, in_=xT_r[:, :, ts(tci, 512)])
                    for pi, (wsb, dest) in enumerate(
                            [(wq_sb, QT), (wk_sb, KT), (wv_sb, None)]):
                        pst = qkvtr_ps.tile([EC, 512], f32, tag="ps1")
                        for dti in range(8):
                            nc.tensor.matmul(
                                pst[:, :], wsb[:, dti, :], xc[:, dti, :],
                                start=(dti == 0), stop=(dti == 7))
                        if dest is not None:
                            nc.scalar.add(
                                dest[:, ts(tci, 512)], pst[:, :],
                                bias_sb[:, pi:pi + 1])
                        else:
                            vtc = vt_tmp.tile([EC, 512], bf16, tag="vtc")
                            nc.scalar.add(vtc[:, :], pst[:, :],
                                          bias_sb[:, 2:3])
                            for ki in range(4):
                                ktg = tci * 4 + ki
                                for h in range(HPC):
                                    trp = qkvtr_ps.tile([128, DH], bf16,
                                                       tag="ps1")
                                    nc.tensor.transpose(
                                        trp[:, :],
                                        vtc[h * DH:(h + 1) * DH, ts(ki, 128)],
                                        ident[h * DH:(h + 1) * DH,
                                              h * DH:(h + 1) * DH])
                                    nc.vector.tensor_copy(
                                        V_sb[:, ktg, h, 0:DH], trp[:, :])

                def emit_attn_pass(b, qh, h):
                    q0 = qh * QHALF
                    qs = slice(b * S + q0, b * S + q0 + QHALF)
                    bh = b * HPC + h
                    hs = slice(h * DH, (h + 1) * DH)
                    # A: scoresT = (K @ Q.T) in kt-pairs + exp
                    exp_tiles = []
                    for ktp in range(NKT // 2):
                        scp = pools["sc_ps"].tile([128, 2, QHALF], f32, tag="scp")
                        for k2 in range(2):
                            kti = 2 * ktp + k2
                            nc.tensor.matmul(
                                scp[:, k2, :],
                                KT[hs, b * S + kti * 128:
                                   b * S + (kti + 1) * 128],
                                QT[hs, qs])
                        et = pools["expp"].tile([128, 2, QHALF], bf16, tag="et")
                        nc.scalar.activation(
                            et[:, :, :], scp[:, :, :], EXP, scale=0.125)
                        exp_tiles.append(et)
                        # raw exp straight out; host normalizes
                        nc.sync.dma_start(
                            out=wT_out[
                                bh, ktp * 256:(ktp + 1) * 256,
                                q0:q0 + QHALF].rearrange(
                                    "(two p) q -> p two q", p=128),
                            in_=et[:, :, :])
                    # B: A@V with ones-column -> sums in row DH
                    avp = pools["av_ps"].tile([DH + 1, QHALF], f32, tag="avp")
                    for kti in range(NKT):
                        nc.tensor.matmul(
                            avp[:, :],
                            V_sb[:, b * NKT + kti, h, :],
                            exp_tiles[kti // 2][:, kti % 2, :],
                            start=(kti == 0), stop=(kti == NKT - 1))
                    # custom-DVE ops mishandle non-zero base partitions;
                    # stage the sums row at partition 0
                    sumrow = pools["invp"].tile([1, QHALF], f32, tag="sumrow")
                    nc.vector.tensor_copy(sumrow[0:1, :], avp[DH:DH + 1, :])
                    invrow = pools["invp"].tile([1, QHALF], f32, tag="invrow")
                    nc.vector.reciprocal_approx_fast(
                        invrow[0:1, :], sumrow[0:1, :])
                    nc.sync.dma_start(
                        out=inv_out[bh, q0:q0 + QHALF].unsqueeze(0),
                        in_=invrow[0:1, :])
                    # C: broadcast 1/sum to the out partitions
                    invbc = pools["invbcp"].tile([DH, QHALF], f32, tag="invbc")
                    nc.gpsimd.partition_broadcast(invbc[:, :], invrow[0:1, :])
                    # D: normalized attention output
                    nc.vector.tensor_mul(
                        outT[:, bh, q0:q0 + QHALF],
                        avp[0:DH, :], invbc[0:DH, :])

                def emit_fc_chunk(b, qh):
                    q0 = qh * QHALF
                    qs = slice(b * S + q0, b * S + q0 + QHALF)
                    for eo in range(D // 128):
                        fcp = pools["fc_ps"].tile([128, QHALF], f32, tag="fcp")
                        for h in range(HPC):
                            nc.tensor.matmul(
                                fcp[:, :],
                                wfc_sb[:, h, ts(eo, 128)],
                                outT[:, b * HPC + h, q0:q0 + QHALF],
                                start=(h == 0), stop=(h == HPC - 1))
                        fcs = pools["fc_sb"].tile([128, QHALF], f32, tag="fcs")
                        nc.vector.tensor_copy(fcs[:, :], fcp[:, :])
                        nc.sync.dma_start(
                            out=fc_out[ts(eo, 128), qs], in_=fcs[:, :])

                NQH = S // QHALF
                for tci in range(T // 512):
                    emit_qkv_chunk(tci)
                phase1.close()
                with (
                    tc.tile_pool(name="expp", bufs=NKT + 4) as expp_,
                    tc.tile_pool(name="invp", bufs=2) as invp_,
                    tc.tile_pool(name="invbcp", bufs=2) as invbcp_,
                    tc.tile_pool(name="fc_sb", bufs=3) as fc_sb_,
                    tc.tile_pool(name="sc_ps", bufs=2, space="PSUM")
                    as sc_ps_,
                    tc.tile_pool(name="av_ps", bufs=2, space="PSUM")
                    as av_ps_,
                    tc.tile_pool(name="fc_ps", bufs=2, space="PSUM")
                    as fc_ps_,
                ):
                    pools.update(expp=expp_, invp=invp_, invbcp=invbcp_,
                                 fc_sb=fc_sb_, sc_ps=sc_ps_, av_ps=av_ps_,
                                 fc_ps=fc_ps_)
                    holdback = [(0, 0), (0, 1)]
                    for b in range(B):
                        for qh in range(NQH):
                            for h in range(HPC):
                                emit_attn_pass(b, qh, h)
                            if (b, qh) not in holdback:
                                emit_fc_chunk(b, qh)
                    # held-back fc chunks fill the PE while the last
                    # attention chain and weight DMAs drain
                    for b, qh in holdback:
                        emit_fc_chunk(b, qh)

    nc.compile()
    return nc


def make_in_maps(x, Wq, bq, Wk, bk, Wv, bv, Wfc, bfc):
    bf = ml_dtypes.bfloat16
    x = np.asarray(x, dtype=np.float32)
    xT = np.ascontiguousarray(x.reshape(T, D).T).astype(bf)  # [D, T]
    Wq = np.asarray(Wq, np.float32)
    Wk = np.asarray(Wk, np.float32)
    Wv = np.asarray(Wv, np.float32)
    Wfc = np.asarray(Wfc, np.float32)
    in_maps = []
    for c in range(N_CORES):
        cs = slice(c * EC, (c + 1) * EC)
        in_maps.append({
            "xT": xT,
            "wqT": np.ascontiguousarray(Wq[cs, :].T).astype(bf),
            "wkT": np.ascontiguousarray(Wk[cs, :].T).astype(bf),
            "wvT": np.ascontiguousarray(Wv[cs, :].T).astype(bf),
            "bqv": np.ascontiguousarray(np.stack(
                [np.asarray(bq, np.float32)[cs],
                 np.asarray(bk, np.float32)[cs],
                 np.asarray(bv, np.float32)[cs]], axis=1)),
            "wfcT": np.ascontiguousarray(
                Wfc[:, cs].T.reshape(HPC, DH, D)).astype(bf),
        })
    return in_maps


def kernel(x, Wq, bq, Wk, bk, Wv, bv, Wfc, bfc):
    from concourse.bass_utils import run_bass_kernel_spmd

    global _NC_CACHE
    if _NC_CACHE is None:
        _NC_CACHE = _build()
    nc = _NC_CACHE

    in_maps = make_in_maps(x, Wq, bq, Wk, bk, Wv, bv, Wfc, bfc)
    res = run_bass_kernel_spmd(nc, in_maps, core_ids=list(range(N_CORES)))

    weights = np.empty((B, NH, S, S), dtype=np.float32)
    fc_acc = np.zeros((D, T), dtype=np.float32)
    for c, r in enumerate(res.results):
        expT = r["wT_out"]                              # [4, S(k), S(q)] bf16
        inv = np.asarray(r["inv_out"], np.float32)      # [4, S(q)]
        for b in range(B):
            for h in range(HPC):
                bh = b * HPC + h
                weights[b, HPC * c + h] = (
                    expT[bh].T.astype(np.float32) * inv[bh][:, None])
        fc_acc += r["fc_out"]

    out = fc_acc.T.reshape(B, S, D) + np.asarray(bfc, np.float32)
    return out, weights


# revision 15
# speedup vs baseline: 1.3584x; 1.3584x over previous
"""Multi-head attention (B=2, S=2048, D=1024, H=16) on 8 Trainium2 NeuronCores.

Strategy: tensor-parallel over heads — 2 heads per core. Each core:
  - computes Q^T/K^T/V for its 2 heads from the full token stream
    (bf16 matmuls with fp32 PSUM accumulation; bf16 keeps the PE HAM
    activity monitor warm at 2.4 GHz — fp32/f32r matmuls run in PE
    transpose-mode which HAM ignores, throttling the clock to 1.2 GHz),
  - computes transposed attention scores  scoresT[k, q] = (K Q^T)/8 per
    (batch, head), exponentiates on the scalar engine (no max-subtraction:
    scores are O(1) for this distribution, exp cannot overflow),
  - gets softmax denominators for free from a ones-column appended to the
    V stationary during the A@V matmul accumulation,
  - ships the raw bf16 exp tiles in [k, q] layout plus the per-q
    reciprocal row sums; the host transposes and normalizes,
  - computes its partial fc projection out_part = Wfc[:, own_cols] @ attn_out
    (the tensor-parallel all-reduce is done on the host at unshard time).

Host side: shard/pre-transpose/bf16-cast inputs, run SPMD on 8 cores, then
  weights[b, 2c+h] = expT_c[2b+h].T * inv_c[2b+h][:, None]
  out = (sum_c fc_part_c).T + bfc.
"""

import numpy as np
import ml_dtypes

B = 2
S = 2048
D = 1024
NH = 16
DH = 64
T = B * S               # 4096 tokens
N_CORES = 8
HPC = NH // N_CORES     # 2 heads per core
EC = HPC * DH           # 128 embedding cols per core
QHALF = 512             # q positions per inner attention pass
NKT = S // 128          # 16 k-tiles per batch

_NC_CACHE = None


def _build():
    from concourse import bacc, mybir
    import concourse.tile as tile
    from concourse.bass import ts
    from concourse.masks import make_identity

    f32 = mybir.dt.float32
    bf16 = mybir.dt.bfloat16
    EXP = mybir.ActivationFunctionType.Exp

    nc = bacc.Bacc("TRN2", target_bir_lowering=False, debug=False,
                   num_devices=N_CORES)

    xT = nc.dram_tensor("xT", [D, T], bf16, kind="ExternalInput").ap()
    wqT = nc.dram_tensor("wqT", [D, EC], bf16, kind="ExternalInput").ap()
    wkT = nc.dram_tensor("wkT", [D, EC], bf16, kind="ExternalInput").ap()
    wvT = nc.dram_tensor("wvT", [D, EC], bf16, kind="ExternalInput").ap()
    bqv = nc.dram_tensor("bqv", [EC, 3], f32, kind="ExternalInput").ap()
    wfcT = nc.dram_tensor("wfcT", [HPC, DH, D], bf16,
                          kind="ExternalInput").ap()

    # raw exp(scores/8) in transposed [k, q] layout: [b*2+h][k][q]
    wT_out = nc.dram_tensor("wT_out", [B * HPC, S, S], bf16,
                            kind="ExternalOutput").ap()
    # reciprocal softmax denominators per q: [b*2+h][q]
    inv_out = nc.dram_tensor("inv_out", [B * HPC, S], f32,
                             kind="ExternalOutput").ap()
    fc_out = nc.dram_tensor("fc_out", [D, T], f32, kind="ExternalOutput").ap()

    xT_r = xT.rearrange("(a p) t -> p a t", p=128)      # [128, 8, T]
    wqT_r = wqT.rearrange("(a p) e -> p a e", p=128)    # [128, 8, EC]
    wkT_r = wkT.rearrange("(a p) e -> p a e", p=128)
    wvT_r = wvT.rearrange("(a p) e -> p a e", p=128)
    wfcT_r = wfcT.rearrange("h e o -> e h o")           # [64, 2, D]

    with tile.TileContext(nc) as tc:
        with (
            tc.tile_pool(name="const", bufs=1) as constp,
            tc.tile_pool(name="persist", bufs=1) as persist,
        ):
            # ---- constants ----
            wq_sb = constp.tile([128, 8, EC], bf16)
            wk_sb = constp.tile([128, 8, EC], bf16)
            wv_sb = constp.tile([128, 8, EC], bf16)
            nc.sync.dma_start(out=wq_sb, in_=wqT_r)
            nc.sync.dma_start(out=wk_sb, in_=wkT_r)
            nc.sync.dma_start(out=wv_sb, in_=wvT_r)
            bias_sb = constp.tile([EC, 3], f32)
            nc.sync.dma_start(out=bias_sb, in_=bqv)
            wfc_sb = constp.tile([DH, HPC, D], bf16)
            nc.sync.dma_start(out=wfc_sb, in_=wfcT_r)
            ident = constp.tile([128, 128], bf16)
            make_identity(nc, ident[:, :])

            # ---- persistent activations ----
            QT = persist.tile([EC, T], bf16)            # [2h*64, t]
            KT = persist.tile([EC, T], bf16)
            # V per global k-tile and head, with a ones column for the
            # softmax denominators: [k(128), kt(32), h(2), DH+1]
            V_sb = persist.tile([128, 2 * NKT, HPC, DH + 1], bf16)
            # attention output (normalized), per (b, h): rows 0..63
            outT = persist.tile([DH, B * HPC, S], bf16)

            nc.vector.memset(V_sb[:, :, :, DH:DH + 1], 1.0)

            # ============ QKV, then attention with fc interleaved ============
            import contextlib
            with contextlib.ExitStack() as phase1:
                xload = phase1.enter_context(
                    tc.tile_pool(name="xload", bufs=3))
                vt_tmp = phase1.enter_context(
                    tc.tile_pool(name="vt_tmp", bufs=2))
                qkv_ps = phase1.enter_context(
                    tc.tile_pool(name="qkv_ps", bufs=2, space="PSUM"))
                tr_ps = phase1.enter_context(
                    tc.tile_pool(name="tr_ps", bufs=2, space="PSUM"))
                pools = {}

                def emit_qkv_chunk(tci):
                    xc = xload.tile([128, 8, 512], bf16)
                    nc.sync.dma_start(out=xc, in_=xT_r[:, :, ts(tci, 512)])
                    for pi, (wsb, dest) in enumerate(
                            [(wq_sb, QT), (wk_sb, KT), (wv_sb, None)]):
                        pst = qkv_ps.tile([EC, 512], f32, tag="pst")
                        for dti in range(8):
                            nc.tensor.matmul(
                                pst[:, :], wsb[:, dti, :], xc[:, dti, :],
                                start=(dti == 0), stop=(dti == 7))
                        if dest is not None:
                            nc.scalar.add(
                                dest[:, ts(tci, 512)], pst[:, :],
                                bias_sb[:, pi:pi + 1])
                        else:
                            vtc = vt_tmp.tile([EC, 512], bf16, tag="vtc")
                            nc.scalar.add(vtc[:, :], pst[:, :],
                                          bias_sb[:, 2:3])
                            for ki in range(4):
                                ktg = tci * 4 + ki
                                for h in range(HPC):
                                    trp = tr_ps.tile([128, DH], bf16,
                                                     tag="trp")
                                    nc.tensor.transpose(
                                        trp[:, :],
                                        vtc[h * DH:(h + 1) * DH, ts(ki, 128)],
                                        ident[h * DH:(h + 1) * DH,
                                              h * DH:(h + 1) * DH])
                                    nc.vector.tensor_copy(
                                        V_sb[:, ktg, h, 0:DH], trp[:, :])

                def emit_attn_pass(b, qh, h):
                    q0 = qh * QHALF
                    qs = slice(b * S + q0, b * S + q0 + QHALF)
                    bh = b * HPC + h
                    hs = slice(h * DH, (h + 1) * DH)
                    # A: scoresT = (K @ Q.T) in kt-pairs + exp
                    exp_tiles = []
                    for ktp in range(NKT // 2):
                        scp = pools["sc_ps"].tile([128, 2, QHALF], f32, tag="scp")
                        for k2 in range(2):
                            kti = 2 * ktp + k2
                            nc.tensor.matmul(
                                scp[:, k2, :],
                                KT[hs, b * S + kti * 128:
                                   b * S + (kti + 1) * 128],
                                QT[hs, qs])
                        et = pools["expp"].tile([128, 2, QHALF], bf16, tag="et")
                        nc.scalar.activation(
                            et[:, :, :], scp[:, :, :], EXP, scale=0.125)
                        exp_tiles.append(et)
                        # raw exp straight out; host normalizes
                        nc.sync.dma_start(
                            out=wT_out[
                                bh, ktp * 256:(ktp + 1) * 256,
                                q0:q0 + QHALF].rearrange(
                                    "(two p) q -> p two q", p=128),
                            in_=et[:, :, :])
                    # B: A@V with ones-column -> sums in row DH
                    avp = pools["av_ps"].tile([DH + 1, QHALF], f32, tag="avp")
                    for kti in range(NKT):
                        nc.tensor.matmul(
                            avp[:, :],
                            V_sb[:, b * NKT + kti, h, :],
                            exp_tiles[kti // 2][:, kti % 2, :],
                            start=(kti == 0), stop=(kti == NKT - 1))
                    # custom-DVE ops mishandle non-zero base partitions;
                    # stage the sums row at partition 0
                    sumrow = pools["invp"].tile([1, QHALF], f32, tag="sumrow")
                    nc.vector.tensor_copy(sumrow[0:1, :], avp[DH:DH + 1, :])
                    invrow = pools["invp"].tile([1, QHALF], f32, tag="invrow")
                    nc.vector.reciprocal_approx_fast(
                        invrow[0:1, :], sumrow[0:1, :])
                    nc.sync.dma_start(
                        out=inv_out[bh, q0:q0 + QHALF].unsqueeze(0),
                        in_=invrow[0:1, :])
                    # C: broadcast 1/sum to the out partitions
                    invbc = pools["invbcp"].tile([DH, QHALF], f32, tag="invbc")
                    nc.gpsimd.partition_broadcast(invbc[:, :], invrow[0:1, :])
                    # D: normalized attention output
                    nc.vector.tensor_mul(
                        outT[:, bh, q0:q0 + QHALF],
                        avp[0:DH, :], invbc[0:DH, :])

                def emit_fc_chunk(b, qh):
                    q0 = qh * QHALF
                    qs = slice(b * S + q0, b * S + q0 + QHALF)
                    for eo in range(D // 128):
                        fcp = pools["fc_ps"].tile([128, QHALF], f32, tag="fcp")
                        for h in range(HPC):
                            nc.tensor.matmul(
                                fcp[:, :],
                                wfc_sb[:, h, ts(eo, 128)],
                                outT[:, b * HPC + h, q0:q0 + QHALF],
                                start=(h == 0), stop=(h == HPC - 1))
                        fcs = pools["fc_sb"].tile([128, QHALF], f32, tag="fcs")
                        nc.vector.tensor_copy(fcs[:, :], fcp[:, :])
                        nc.sync.dma_start(
                            out=fc_out[ts(eo, 128), qs], in_=fcs[:, :])

                NQH = S // QHALF
                for tci in range(T // 512):
                    emit_qkv_chunk(tci)
                phase1.close()
                with (
                    tc.tile_pool(name="expp", bufs=NKT + 4) as expp_,
                    tc.tile_pool(name="invp", bufs=2) as invp_,
                    tc.tile_pool(name="invbcp", bufs=2) as invbcp_,
                    tc.tile_pool(name="fc_sb", bufs=3) as fc_sb_,
                    tc.tile_pool(name="sc_ps", bufs=2, space="PSUM")
                    as sc_ps_,
                    tc.tile_pool(name="av_ps", bufs=2, space="PSUM")
                    as av_ps_,
                    tc.tile_pool(name="fc_ps", bufs=2, space="PSUM")
                    as fc_ps_,
                ):
                    pools.update(expp=expp_, invp=invp_, invbcp=invbcp_,
                                 fc_sb=fc_sb_, sc_ps=sc_ps_, av_ps=av_ps_,
                                 fc_ps=fc_ps_)
                    holdback = [(0, 0), (0, 1)]
                    for b in range(B):
                        for qh in range(NQH):
                            for h in range(HPC):
                                emit_attn_pass(b, qh, h)
                            if (b, qh) not in holdback:
                                emit_fc_chunk(b, qh)
                    # held-back fc chunks fill the PE while the last
                    # attention chain and weight DMAs drain
                    for b, qh in holdback:
                        emit_fc_chunk(b, qh)

    nc.compile()
    return nc


def make_in_maps(x, Wq, bq, Wk, bk, Wv, bv, Wfc, bfc):
    bf = ml_dtypes.bfloat16
    x = np.asarray(x, dtype=np.float32)
    xT = np.ascontiguousarray(x.reshape(T, D).T).astype(bf)  # [D, T]
    Wq = np.asarray(Wq, np.float32)
    Wk = np.asarray(Wk, np.float32)
    Wv = np.asarray(Wv, np.float32)
    Wfc = np.asarray(Wfc, np.float32)
    in_maps = []
    for c in range(N_CORES):
        cs = slice(c * EC, (c + 1) * EC)
        in_maps.append({
            "xT": xT,
            "wqT": np.ascontiguousarray(Wq[cs, :].T).astype(bf),
            "wkT": np.ascontiguousarray(Wk[cs, :].T).astype(bf),
            "wvT": np.ascontiguousarray(Wv[cs, :].T).astype(bf),
            "bqv": np.ascontiguousarray(np.stack(
                [np.asarray(bq, np.float32)[cs],
                 np.asarray(bk, np.float32)[cs],
                 np.asarray(bv, np.float32)[cs]], axis=1)),
            "wfcT": np.ascontiguousarray(
                Wfc[:, cs].T.reshape(HPC, DH, D)).astype(bf),
        })
    return in_maps


def kernel(x, Wq, bq, Wk, bk, Wv, bv, Wfc, bfc):
    from concourse.bass_utils import run_bass_kernel_spmd

    global _NC_CACHE
    if _NC_CACHE is None:
        _NC_CACHE = _build()
    nc = _NC_CACHE

    in_maps = make_in_maps(x, Wq, bq, Wk, bk, Wv, bv, Wfc, bfc)
    res = run_bass_kernel_spmd(nc, in_maps, core_ids=list(range(N_CORES)))

    weights = np.empty((B, NH, S, S), dtype=np.float32)
    fc_acc = np.zeros((D, T), dtype=np.float32)
    for c, r in enumerate(res.results):
        expT = r["wT_out"]                              # [4, S(k), S(q)] bf16
        inv = np.asarray(r["inv_out"], np.float32)      # [4, S(q)]
        for b in range(B):
            for h in range(HPC):
                bh = b * HPC + h
                weights[b, HPC * c + h] = (
                    expT[bh].T.astype(np.float32) * inv[bh][:, None])
        fc_acc += r["fc_out"]

    out = fc_acc.T.reshape(B, S, D) + np.asarray(bfc, np.float32)
    return out, weights
